# revision 18
# baseline (speedup 1.0000x reference)
import sys

if "/opt/trn_rl_repo" not in sys.path:
    sys.path.insert(0, "/opt/trn_rl_repo")

import numpy as np

import concourse.bacc as bacc
import concourse.mybir as mybir
import concourse.tile as tile
from concourse.bass_utils import run_bass_kernel_spmd

dt = mybir.dt
Alu = mybir.AluOpType

TOKENS = 8192
IN_FEATURES = 4096
OUT_FEATURES = 4096

N_CORES = 8
T_SHARD = 2
F_SHARD = 4

TOK = TOKENS // T_SHARD
K = IN_FEATURES
FPC = OUT_FEATURES // F_SHARD
KB = K // 2
KC = K // 128
FT = FPC // 128
MT = TOK // 128
NG = FPC // 512
KC2 = 16
KC1 = KC - KC2
CH = KB // 2
KH = KC1 // 2

BATCH = 8


def build(reps=1):
    nc = bacc.Bacc()
    x_d = nc.dram_tensor("x", [K, TOK], dt.float32, kind="ExternalInput")
    wp_d = nc.dram_tensor("wp", [FPC, KB], dt.uint16, kind="ExternalInput")
    ws_d = nc.dram_tensor("ws", [1], dt.float32, kind="ExternalInput")
    bias_d = nc.dram_tensor("bias", [FPC], dt.float32, kind="ExternalInput")
    out_d = nc.dram_tensor("out", [TOK, FPC], dt.float32, kind="ExternalOutput")

    with tile.TileContext(nc) as tc:
        with (
            tc.tile_pool(name="const", bufs=1) as const,
            tc.tile_pool(name="wdq", bufs=1) as wdq_pool,
            tc.tile_pool(name="xpool", bufs=2) as xpool,
            tc.tile_pool(name="opool", bufs=4) as opool,
            tc.tile_pool(name="psum", bufs=BATCH, space="PSUM") as psum_pool,
        ):
            scol = const.tile([128, 1], dt.float32)
            nc.sync.dma_start(
                scol[:], ws_d[:].rearrange("(a s) -> a s", a=1).to_broadcast([128, 1])
            )
            bt = const.tile([128, FPC], dt.float32)
            nc.sync.dma_start(
                bt[:],
                bias_d[:].rearrange("(a f) -> a f", a=1).to_broadcast([128, FPC]),
            )

            for _rep in range(reps):
                wt_g = [
                    const.tile([128, 4, KC1, 128], dt.bfloat16, name=f"wtg{g}")
                    for g in range(NG)
                ]
                wt8_g = [
                    const.tile([128, KC2, 512], dt.float8e4, name=f"wt8g{g}")
                    for g in range(NG)
                ]

                xt_tiles = {}

                def issue_xa(m):
                    xt = xpool.tile([128, KC1, 128], dt.bfloat16, name="xt", bufs=9)
                    nc.gpsimd.dma_start(
                        xt[:],
                        x_d[: K // 2, m * 128 : (m + 1) * 128].rearrange(
                            "(kc p) t -> p kc t", p=128
                        ),
                    )
                    xt_tiles[m] = [xt, None]

                def issue_xb(m):
                    xbf = xpool.tile([128, KC2, 128], dt.float32, name="xbf", bufs=3)
                    nc.sync.dma_start(
                        xbf[:],
                        x_d[K // 2 :, m * 128 : (m + 1) * 128].rearrange(
                            "(kc p) t -> p kc t", p=128
                        ),
                    )
                    xt8 = xpool.tile([128, KC2, 128], dt.float8e4, name="xt8", bufs=8)
                    nc.vector.tensor_copy(xt8[:], xbf[:])
                    xt_tiles[m][1] = xt8

                b_all = [None] * FT

                def load_wp(ft):
                    b = wdq_pool.tile([128, 2 * CH], dt.uint16, name=f"ball{ft}")
                    nc.sync.dma_start(b[:], wp_d[ft * 128 : (ft + 1) * 128, :])
                    b_all[ft] = b

                for ft in range(4):
                    load_wp(ft)

                def emit_half(v, tag, ft, half):
                    b32 = b_all[ft][:, half * CH : (half + 1) * CH].bitcast(dt.int32)
                    e = wdq_pool.tile([128, CH], dt.uint16, name=f"e{tag}", bufs=3)
                    o = wdq_pool.tile([128, CH], dt.uint16, name=f"o{tag}", bufs=3)
                    s = wdq_pool.tile([128, CH], dt.uint16, name=f"s{tag}")
                    c = wdq_pool.tile([128, CH], dt.bfloat16, name=f"c{tag}")
                    s32 = s[:].bitcast(dt.int32)
                    for tgt, mask, shl, smask, sshl in (
                        (e, 0x00700070, 2, 0x00800080, 8),
                        (o, 0x00070007, 6, 0x00080008, 12),
                    ):
                        t32 = tgt[:].bitcast(dt.int32)
                        tbf = tgt[:].bitcast(dt.bfloat16)
                        v.tensor_scalar(
                            t32, b32, mask, shl,
                            Alu.bitwise_and, Alu.logical_shift_left,
                        )
                        v.tensor_scalar(tgt[:], tgt[:], 0x3F00, None, Alu.add)
                        v.tensor_scalar(c[:], tbf, 1.0, 0.0, Alu.subtract, Alu.min)
                        v.tensor_tensor(tbf, tbf, c[:], Alu.add)
                        v.tensor_scalar(
                            s32, b32, smask, sshl,
                            Alu.bitwise_and, Alu.logical_shift_left,
                        )
                        v.tensor_tensor(t32, t32, s32, Alu.bitwise_or)
                    wtg, fl = wt_g[ft // 4], ft % 4
                    if half == 0:
                        nc.scalar.dma_start_transpose(
                            wtg[:, fl, :KH, :], e[:].bitcast(dt.bfloat16)
                        )
                        nc.scalar.dma_start_transpose(
                            wtg[:, fl, KH:, :], o[:].bitcast(dt.bfloat16)
                        )
                    else:
                        for src_t, c0 in ((e, 0), (o, KH)):
                            wtb = wdq_pool.tile(
                                [128, KH, 128], dt.bfloat16, name=f"wtb{c0}", bufs=2
                            )
                            nc.scalar.dma_start_transpose(
                                wtb[:], src_t[:].bitcast(dt.bfloat16)
                            )
                            nc.scalar.copy(
                                wt8_g[ft // 4][
                                    :, c0 : c0 + KH, fl * 128 : (fl + 1) * 128
                                ],
                                wtb[:],
                            )

                emit_half(nc.vector, "v", 0, 0)
                emit_half(nc.vector, "v", 1, 0)
                issue_xa(0)
                issue_xa(1)
                emit_half(nc.vector, "v", 2, 0)
                emit_half(nc.vector, "v", 3, 0)
                issue_xa(2)
                issue_xa(3)
                for ft in range(4, FT):
                    load_wp(ft)
                emit_half(nc.vector, "v", 0, 1)
                issue_xa(4)
                issue_xa(5)
                emit_half(nc.vector, "v", 1, 1)
                issue_xb(0)
                issue_xb(1)
                issue_xa(6)
                issue_xa(7)
                emit_half(nc.vector, "v", 2, 1)
                issue_xb(2)
                issue_xb(3)
                emit_half(nc.vector, "v", 3, 1)
                issue_xb(4)
                issue_xb(5)
                for ft in range(4, FT):
                    emit_half(nc.vector, "v", ft, 0)
                issue_xb(6)
                issue_xb(7)
                for ft in range(4, FT):
                    emit_half(nc.vector, "v", ft, 1)

                for b0 in range(0, MT, BATCH):
                    ms = list(range(b0, min(b0 + BATCH, MT)))
                    for g in range(NG):
                        pss = []
                        for i, m in enumerate(ms):
                            if g == 1 and b0 + BATCH + i < MT:
                                issue_xa(b0 + BATCH + i)
                                issue_xb(b0 + BATCH + i)
                            xt = xt_tiles[m][0]
                            ps = psum_pool.tile([128, 512], dt.float32)
                            for kc in range(KC1):
                                nc.tensor.matmul(
                                    ps[:],
                                    xt[:, kc, :],
                                    wt_g[g][:, :, kc, :],
                                    start=(kc == 0),
                                    stop=False,
                                )
                            pss.append(ps)
                        for m, ps in zip(ms, pss):
                            xt8 = xt_tiles[m][1]
                            for cc in range(KC2 // 2):
                                nc.tensor.matmul(
                                    ps[:],
                                    xt8[:, 2 * cc : 2 * cc + 2, :],
                                    wt8_g[g][:, 2 * cc : 2 * cc + 2, :],
                                    start=False,
                                    stop=(cc == KC2 // 2 - 1),
                                    perf_mode=mybir.MatmulPerfMode.DoubleRow,
                                )
                            osb = opool.tile([128, 512], dt.float32, name="osb")
                            nc.vector.scalar_tensor_tensor(
                                osb[:],
                                ps[:],
                                scol[:],
                                bt[:, g * 512 : (g + 1) * 512],
                                Alu.mult,
                                Alu.add,
                            )
                            nc.sync.dma_start(
                                out_d[
                                    m * 128 : (m + 1) * 128,
                                    g * 512 : (g + 1) * 512,
                                ],
                                osb[:],
                            )
    nc.finalize()
    return nc


_NC = None


def _get_nc():
    global _NC
    if _NC is None:
        _NC = build()
    return _NC


_KPERM = np.concatenate(
    [
        np.arange(0, K // 2, 2),
        np.arange(1, K // 2, 2),
        np.arange(K // 2, K, 2),
        np.arange(K // 2 + 1, K, 2),
    ]
)


def make_in_maps(x, weight_packed, weight_scale, bias):
    x = np.asarray(x, dtype=np.float32)
    wp = np.asarray(weight_packed, dtype=np.int32).reshape(OUT_FEATURES, KB)
    wp16 = wp.astype(np.uint16)
    ws = np.ascontiguousarray(np.asarray(weight_scale, dtype=np.float32))
    bias = np.asarray(bias, dtype=np.float32)
    xT = [
        np.ascontiguousarray(x[th * TOK : (th + 1) * TOK].T[_KPERM])
        for th in range(T_SHARD)
    ]
    in_maps = []
    for core in range(N_CORES):
        th, q = divmod(core, F_SHARD)
        in_maps.append(
            {
                "x": xT[th],
                "wp": np.ascontiguousarray(wp16[q * FPC : (q + 1) * FPC]),
                "ws": ws,
                "bias": np.ascontiguousarray(bias[q * FPC : (q + 1) * FPC]),
            }
        )
    return in_maps


def unshard(results):
    out = np.empty((TOKENS, OUT_FEATURES), dtype=np.float32)
    for core in range(N_CORES):
        th, q = divmod(core, F_SHARD)
        out[th * TOK : (th + 1) * TOK, q * FPC : (q + 1) * FPC] = results[core]["out"]
    return out


def run(inputs, **kwargs):
    nc = _get_nc()
    res = run_bass_kernel_spmd(
        nc, make_in_maps(**inputs), core_ids=list(range(N_CORES)), **kwargs
    )
    return unshard(res.results), res


def kernel(x, weight_packed, weight_scale, bias):
    out, _ = run(
        {
            "x": x,
            "weight_packed": weight_packed,
            "weight_scale": weight_scale,
            "bias": bias,
        }
    )
    return out


if __name__ == "__main__":
    rng = np.random.default_rng(0)
    inputs = {
        "x": rng.standard_normal((TOKENS, IN_FEATURES), dtype=np.float32),
        "weight_packed": rng.integers(
            0, 256, size=OUT_FEATURES * IN_FEATURES // 2
        ).astype(np.int32),
        "weight_scale": rng.random(1, dtype=np.float32),
        "bias": rng.standard_normal(OUT_FEATURES).astype(np.float32),
    }
    out = kernel(**inputs)
    print("out", out.shape, out.dtype, out[0, :4])
